# revision 1
# baseline (speedup 1.0000x reference)
"""Trainium2 Bass kernel for nn_DeconvCG (bilateral grid splat->blur->slice).

12 independent (batch,channel) images -> 24 half-images, 3 per NeuronCore
(pure data parallel, no collectives).

Per half-image:
  SPLAT: per-pixel bin one-hot (bf16) + delta = fz-round(fz) (bf16); the
    y-reduction (rows->cells, exact banker's rounding baked into a static 0/1
    matrix Sy) runs on the TensorEngine as bf16 matmuls with fp32 PSUM
    accumulation; x-reduction via grouped tensor_reduce + banker's-rounding
    corrections.  val_b = (b*cnt_b + sum(delta))/15 recovers exact value sums.
  BLUR: z/x 5-tap correlations on the small grid (taps are compile-time
    constants: filters are visible host-side); the y-blur is folded into the
    slice's y-interpolation matrix (host-computed).
  SLICE: y-expand on the PE; per-pixel z-gather of the (z0,z0+1) pair via a
    copy_predicated chain; x-lerp via x-difference grids read through
    stride-0 replicated access patterns (no data expansion).
"""
import sys

import numpy as np
import ml_dtypes

sys.path.insert(0, "/opt/trn_rl_repo")

import concourse.bass as bass
import concourse.mybir as mybir
import concourse.tile as tile
import concourse.bacc as bacc
from concourse import bass_utils

F32 = mybir.dt.float32
BF16 = mybir.dt.bfloat16
ALU = mybir.AluOpType
AX = mybir.AxisListType
ACTF = mybir.ActivationFunctionType

S = 8
NB = 16
H = W = 1024
GW = 129          # x cells
NCY = 68          # y-cell slots per half (67 used, padded)
NROW = 640        # padded rows per half (5 chunks of 128)
WP = 1032         # padded x: [-4, 1028)
OUT_OFF = 20      # local row of first output row
NCH = 5           # splat row chunks
NQ = 4            # slice row chunks (512 out rows)
MAGIC = 12582912.0  # 1.5 * 2**23


def _round_half_even_cells(rows):
    return np.round(rows.astype(np.float32) / np.float32(S)).astype(np.int64)


def _half_geometry(half):
    o0 = half * 512
    rows_out = np.arange(o0, o0 + 512)
    y0 = rows_out // S
    cyb0, cyb1 = int(y0.min()), int(y0.max() + 1)
    cyr0 = max(cyb0 - 2, 0)
    cyr1 = min(cyb1 + 2, GW - 1)
    return o0, cyr0, cyr1, cyb0, cyb1


def _host_inputs_for_half(img, fs, half):
    """img [1024,1024] f32 -> (padded rows [640,1032] f32,
    Sy [5,128,68] bf16, LyGT [4,68,128] f32)."""
    o0, cyr0, cyr1, cyb0, cyb1 = _half_geometry(half)
    pad = np.full((NROW, WP), -1.0, np.float32)
    g0 = o0 - OUT_OFF
    glo, ghi = max(0, g0), min(H, g0 + NROW)
    pad[glo - g0:ghi - g0, 4:4 + W] = img[glo:ghi]

    sy = np.zeros((NCH, 128, NCY), np.float32)
    for c in range(NCH):
        loc = 128 * c + np.arange(128)
        g = g0 + loc
        valid = (g >= 0) & (g < H)
        cells = _round_half_even_cells(np.clip(g, 0, H - 1))
        ok = valid & (cells >= cyr0) & (cells <= cyr1)
        sy[c, np.arange(128)[ok], cells[ok] - cyr0] = 1.0

    ncyb = cyb1 - cyb0 + 1
    rows_out = np.arange(o0, o0 + 512)
    y0 = rows_out // S
    ty = (rows_out % S).astype(np.float32) / np.float32(S)
    Ly = np.zeros((512, ncyb), np.float32)
    Ly[np.arange(512), y0 - cyb0] = 1 - ty
    Ly[np.arange(512), y0 + 1 - cyb0] = ty
    Gy = np.zeros((ncyb, NCY), np.float32)
    for i in range(5):
        for cb in range(cyb0, cyb1 + 1):
            cr = cb + i - 2
            if cyr0 <= cr <= cyr1:
                Gy[cb - cyb0, cr - cyr0] += fs[i]
    LyG = (Ly @ Gy).astype(np.float32)          # [512, NCY]
    lygt = np.zeros((NQ, NCY, 128), np.float32)
    for q in range(NQ):
        lygt[q] = LyG[128 * q:128 * q + 128].T
    return pad, sy.astype(ml_dtypes.bfloat16), lygt


def _ap(base, off_elems, free_pairs):
    """AP reusing base's partition pair with custom free dims (elem offsets)."""
    return bass.AP(base.tensor, base.offset + off_elems,
                   [list(base.ap[0])] + [list(p) for p in free_pairs])


def build_program(fs, fr):
    nc = bacc.Bacc(None, target_bir_lowering=False)
    halves = nc.dram_tensor("halves", [3, NROW, WP], F32, kind="ExternalInput")
    syd = nc.dram_tensor("sy", [3, NCH, 128, NCY], BF16, kind="ExternalInput")
    lygtd = nc.dram_tensor("lygt", [3, NQ, NCY, 128], F32, kind="ExternalInput")
    txd = nc.dram_tensor("txp", [128, W], F32, kind="ExternalInput")
    outd = nc.dram_tensor("out", [3, 512, W], F32, kind="ExternalOutput")

    with tile.TileContext(nc) as tc:
        with (
            tc.tile_pool(name="img", bufs=2) as imgp,
            tc.tile_pool(name="gzdb", bufs=5) as gzdbp,
            tc.tile_pool(name="syp", bufs=6) as syp,
            tc.tile_pool(name="plane", bufs=4) as planep,
            tc.tile_pool(name="ps", bufs=2, space="PSUM") as psp,
            tc.tile_pool(name="grid", bufs=1) as gridp,
            tc.tile_pool(name="mask", bufs=14) as maskp,
            tc.tile_pool(name="acc", bufs=1) as accp,
            tc.tile_pool(name="tmp", bufs=3) as tmpp,
            tc.tile_pool(name="keep", bufs=1) as keepp,
            tc.tile_pool(name="cst", bufs=1) as cstp,
        ):
            txt = cstp.tile([128, W], F32, tag="tx")
            nc.gpsimd.dma_start(txt[:], txd[:, :])

            for h in range(3):
                # ---------------- SPLAT ----------------
                gzbs, dbs, sys_ = [], [], []
                for c in range(NCH):
                    img = imgp.tile([128, WP], F32, tag="img")
                    nc.gpsimd.dma_start(img[:], halves[h, 128 * c:128 * c + 128, :])
                    fz = imgp.tile([128, WP], F32, tag="sfz")
                    nc.vector.tensor_scalar(fz[:], img[:], 15.0, None, ALU.mult)
                    gz = imgp.tile([128, WP], F32, tag="sfz")
                    nc.vector.tensor_scalar(gz[:], fz[:], MAGIC, MAGIC, ALU.add,
                                            ALU.subtract)
                    gzb = gzdbp.tile([128, WP], BF16, tag="gzb")
                    nc.vector.tensor_copy(gzb[:], gz[:])
                    db = gzdbp.tile([128, WP], BF16, tag="db")
                    nc.vector.tensor_tensor(db[:], fz[:], gz[:], ALU.subtract)
                    syt = syp.tile([128, NCY], BF16, tag="sy")
                    nc.gpsimd.dma_start(syt[:], syd[h, c])
                    gzbs.append(gzb); dbs.append(db); sys_.append(syt)

                # VX: [68, (plane2, z16, cx129)]  plane0 = cnt, plane1 = D->val
                vx = gridp.tile([NCY, 2 * NB * GW], F32, tag="ga")

                for b in range(NB):
                    psC = psp.tile([NCY, WP], F32, tag="ps")
                    psD = psp.tile([NCY, WP], F32, tag="ps")
                    for c in range(NCH):
                        cnt = planep.tile([128, WP], BF16, tag="pl")
                        nc.vector.tensor_scalar(cnt[:], gzbs[c][:], float(b),
                                                None, ALU.is_equal)
                        dpl = planep.tile([128, WP], BF16, tag="pl")
                        nc.vector.scalar_tensor_tensor(dpl[:], gzbs[c][:],
                                                       float(b), dbs[c][:],
                                                       ALU.is_equal, ALU.mult)
                        for (lo, hi) in ((0, 512), (512, 1024), (1024, 1032)):
                            nc.tensor.matmul(psC[:, lo:hi], sys_[c][:],
                                             cnt[:, lo:hi], start=(c == 0),
                                             stop=(c == NCH - 1))
                            nc.tensor.matmul(psD[:, lo:hi], sys_[c][:],
                                             dpl[:, lo:hi], start=(c == 0),
                                             stop=(c == NCH - 1))
                    for (p, ps) in ((0, psC), (1, psD)):
                        dst = _ap(vx[:, :], (p * NB + b) * GW, [[1, GW]])
                        src = _ap(ps[:, :], 0, [[8, GW], [1, 8]])
                        nc.vector.tensor_reduce(dst, src, AX.X, ALU.add)
                        corr = tmpp.tile([NCY, 64], F32, tag="corr")
                        nc.scalar.copy(corr[:], _ap(ps[:, :], 8, [[16, 64]]))
                        odd = _ap(vx[:, :], (p * NB + b) * GW + 1, [[2, 64]])
                        nc.vector.tensor_tensor(odd, odd, corr[:], ALU.subtract)
                        even = _ap(vx[:, :], (p * NB + b) * GW, [[2, 64]])
                        nc.vector.tensor_tensor(even, even, corr[:], ALU.add)

                # decode val_b = b*cnt_b + D_b (in place into plane 1)
                for b in range(NB):
                    cnt = _ap(vx[:, :], b * GW, [[1, GW]])
                    dsl = _ap(vx[:, :], (NB + b) * GW, [[1, GW]])
                    nc.vector.scalar_tensor_tensor(dsl, cnt, float(b), dsl,
                                                   ALU.mult, ALU.add)

                # z-blur VX -> VZ
                vz = gridp.tile([NCY, 2 * NB * GW], F32, tag="gb")
                nc.vector.memset(vz[:], 0.0)
                for i in range(5):
                    sh = i - 2
                    z0r, z1r = max(0, -sh), NB - max(0, sh)
                    nzz = z1r - z0r
                    dst = _ap(vz[:, :], z0r * GW,
                              [[NB * GW, 2], [GW, nzz], [1, GW]])
                    src = _ap(vx[:, :], (z0r + sh) * GW,
                              [[NB * GW, 2], [GW, nzz], [1, GW]])
                    nc.vector.scalar_tensor_tensor(dst, src, float(fr[i]), dst,
                                                   ALU.mult, ALU.add)
                # x-blur VZ -> VB (reuses VX's slot via tag "ga")
                vb = gridp.tile([NCY, 2 * NB * GW], F32, tag="ga")
                nc.vector.memset(vb[:], 0.0)
                for i in range(5):
                    sh = i - 2
                    x0r, x1r = max(0, -sh), GW - max(0, sh)
                    nxx = x1r - x0r
                    dst = _ap(vb[:, :], x0r, [[GW, 2 * NB], [1, nxx]])
                    src = _ap(vz[:, :], x0r + sh, [[GW, 2 * NB], [1, nxx]])
                    nc.vector.scalar_tensor_tensor(dst, src, float(fs[i]), dst,
                                                   ALU.mult, ALU.add)
                # x-diff VD (reuses VZ's slot via tag "gb")
                vd = gridp.tile([NCY, 2 * NB * 128], F32, tag="gb")
                nc.vector.tensor_tensor(
                    _ap(vd[:, :], 0, [[128, 2 * NB], [1, 128]]),
                    _ap(vb[:, :], 1, [[GW, 2 * NB], [1, 128]]),
                    _ap(vb[:, :], 0, [[GW, 2 * NB], [1, 128]]),
                    ALU.subtract)

                # ---------------- SLICE ----------------
                for q in range(NQ):
                    lygt = syp.tile([NCY, 128], F32, tag="lygt")
                    nc.gpsimd.dma_start(lygt[:], lygtd[h, q])
                    img = imgp.tile([128, W], F32, tag="imgo")
                    r0 = OUT_OFF + 128 * q
                    nc.gpsimd.dma_start(img[:], halves[h, r0:r0 + 128, 4:4 + W])
                    fz = imgp.tile([128, W], F32, tag="fzo")
                    nc.vector.tensor_scalar(fz[:], img[:], 15.0, None, ALU.mult)
                    rr = tmpp.tile([128, W], F32, tag="scr")
                    nc.vector.tensor_scalar(rr[:], fz[:], MAGIC, MAGIC, ALU.add,
                                            ALU.subtract)
                    gt = tmpp.tile([128, W], F32, tag="scr")
                    nc.vector.tensor_tensor(gt[:], rr[:], fz[:], ALU.is_gt)
                    z0 = tmpp.tile([128, W], F32, tag="scr")
                    nc.vector.tensor_tensor(z0[:], rr[:], gt[:], ALU.subtract)
                    tz = keepp.tile([128, W], F32, tag="tz")
                    nc.vector.tensor_tensor(tz[:], fz[:], z0[:], ALU.subtract)
                    ges = []
                    for m in range(1, 15):
                        ge = maskp.tile([128, W], mybir.dt.uint8, tag="ge")
                        nc.vector.tensor_scalar(ge[:], z0[:], float(m), None,
                                                ALU.is_ge)
                        ges.append(ge)
                    omtz = keepp.tile([128, W], F32, tag="omtz")
                    nc.vector.tensor_scalar(omtz[:], tz[:], -1.0, 1.0, ALU.mult,
                                            ALU.add)

                    ovs = {}
                    for p in (0, 1):            # 0 = wt, 1 = val
                        accA = accp.tile([128, 2 * W], F32, tag="accA")
                        accD = accp.tile([128, 2 * W], F32, tag="accD")
                        for zh in (0, 1):
                            nz = 9 if zh == 0 else 8
                            zb = 8 * zh
                            psV = psp.tile([128, nz * GW], F32, tag="ps")
                            psD2 = psp.tile([128, nz * 128], F32, tag="ps")
                            for (ps, src, wid) in ((psV, vb, GW),
                                                   (psD2, vd, 128)):
                                ntot = nz * wid
                                base = (p * NB + zb) * wid
                                lo = 0
                                while lo < ntot:
                                    hi = min(lo + 512, ntot)
                                    nc.tensor.matmul(
                                        ps[:, lo:hi], lygt[:],
                                        _ap(src[:, :], base + lo,
                                            [[1, hi - lo]]),
                                        start=True, stop=True)
                                    lo = hi
                            ms = range(0, 8) if zh == 0 else range(8, 15)
                            for m in ms:
                                zl = m - zb
                                dvV = _ap(psV[:, :], zl * GW,
                                          [[1, 128], [0, 8], [GW, 2]])
                                dvD = _ap(psD2[:, :], zl * 128,
                                          [[1, 128], [0, 8], [128, 2]])
                                oA = _ap(accA[:, :], 0,
                                         [[16, 128], [2, 8], [1, 2]])
                                oD = _ap(accD[:, :], 0,
                                         [[16, 128], [2, 8], [1, 2]])
                                if m == 0:
                                    nc.vector.tensor_copy(oA, dvV)
                                    nc.vector.tensor_copy(oD, dvD)
                                else:
                                    mk = _ap(ges[m - 1][:, :], 0,
                                             [[8, 128], [1, 8], [0, 2]])
                                    nc.vector.copy_predicated(oA, mk, dvV)
                                    nc.vector.copy_predicated(oD, mk, dvD)
                        # combine to ov_p = (1-tz)(A + tx*DA) + tz(B + tx*DB)
                        a0 = _ap(accA[:, :], 0, [[2, W]])
                        a1 = _ap(accA[:, :], 1, [[2, W]])
                        d0 = _ap(accD[:, :], 0, [[2, W]])
                        d1 = _ap(accD[:, :], 1, [[2, W]])
                        t1 = tmpp.tile([128, W], F32, tag="sc2")
                        nc.vector.tensor_tensor(t1[:], txt[:], d0, ALU.mult)
                        av = tmpp.tile([128, W], F32, tag="sc2")
                        nc.vector.tensor_tensor(av[:], t1[:], a0, ALU.add)
                        t2 = tmpp.tile([128, W], F32, tag="sc2")
                        nc.vector.tensor_tensor(t2[:], txt[:], d1, ALU.mult)
                        bv = tmpp.tile([128, W], F32, tag="sc2")
                        nc.vector.tensor_tensor(bv[:], t2[:], a1, ALU.add)
                        nc.vector.tensor_tensor(av[:], av[:], omtz[:], ALU.mult)
                        nc.vector.tensor_tensor(bv[:], bv[:], tz[:], ALU.mult)
                        ov = keepp.tile([128, W], F32, tag=f"ov{p}")
                        nc.vector.tensor_tensor(ov[:], av[:], bv[:], ALU.add)
                        ovs[p] = ov
                    den = tmpp.tile([128, W], F32, tag="sc2")
                    nc.vector.tensor_scalar(den[:], ovs[0][:], 15.0, 1.5e-7,
                                            ALU.mult, ALU.add)
                    rec = tmpp.tile([128, W], F32, tag="sc2")
                    scr = tmpp.tile([128, W], F32, tag="sc2")
                    nc.vector.reciprocal_approx_accurate(rec[:], den[:], scr[:])
                    res = tmpp.tile([128, W], F32, tag="sc2")
                    nc.vector.tensor_tensor(res[:], ovs[1][:], rec[:], ALU.mult)
                    nc.gpsimd.dma_start(outd[h, 128 * q:128 * q + 128, :], res[:])
    nc.finalize()
    return nc


_PROGRAM_CACHE = {}


def _cached_program(fs, fr):
    key = (tuple(np.asarray(fs, np.float32).tolist()),
           tuple(np.asarray(fr, np.float32).tolist()))
    if key not in _PROGRAM_CACHE:
        _PROGRAM_CACHE[key] = build_program(np.asarray(fs, np.float32),
                                            np.asarray(fr, np.float32))
    return _PROGRAM_CACHE[key]


def kernel(blurred_batch, kernel_batch, filter_s, filter_r,
           num_irls_iter=None, num_cg_iter=None):
    imgs = np.asarray(blurred_batch, np.float32).reshape(12, H, W)
    fs = np.asarray(filter_s, np.float32)
    fr = np.asarray(filter_r, np.float32)

    tx = np.tile(((np.arange(W) % S) / np.float32(S)).astype(np.float32),
                 (128, 1))

    nc = _cached_program(fs, fr)

    in_maps = []
    for core in range(8):
        hv = np.zeros((3, NROW, WP), np.float32)
        sy = np.zeros((3, NCH, 128, NCY), ml_dtypes.bfloat16)
        ly = np.zeros((3, NQ, NCY, 128), np.float32)
        for s in range(3):
            g = 3 * core + s
            pad, syh, lygt = _host_inputs_for_half(imgs[g // 2], fs, g % 2)
            hv[s], sy[s], ly[s] = pad, syh, lygt
        in_maps.append({"halves": hv, "sy": sy, "lygt": ly, "txp": tx})

    res = bass_utils.run_bass_kernel_spmd(nc, in_maps, core_ids=list(range(8)))
    out = np.zeros((12, H, W), np.float32)
    for core in range(8):
        o = res.results[core]["out"]
        for s in range(3):
            g = 3 * core + s
            out[g // 2, (g % 2) * 512:(g % 2) * 512 + 512] = o[s]
    return out.reshape(4, 3, H, W)



# revision 15
# speedup vs baseline: 2.3176x; 2.3176x over previous
"""Trainium2 Bass kernel for nn_DeconvCG (bilateral grid splat->blur->slice).

12 independent (batch,channel) images -> 24 half-images, 3 per NeuronCore
(pure data parallel, no collectives).

Per half-image:
  SPLAT: fp16 per-bin one-hot masks (is_equal @4x DVE mode) and mask*delta
    planes (@2x); y-reduction rows->cells on the PE (Sy fp16 stationary,
    fp32 PSUM accumulate over 5 row chunks); x-reduction via grouped
    tensor_reduce + banker's-rounding corrections (as before).
  BLUR: z/x 5-tap correlations on the small grid in fp16 via
    (tensor_scalar mult @4x + tensor_tensor add @2x) pairs, center tap
    written first (no memset).
  SLICE: two-pass PE pipeline: pass1 produces the y-interpolated grid
    TRANSPOSED (cells x rows) per (z,plane); pass2 multiplies by the
    compile-time x-lerp matrix Lx to produce pixel-resolution planes E_z.
    The per-pixel z-interp uses the tent identity
        out = sum_z max(0, 1-|fz-z|) * E_z
            = S - sum_z min(|fz-z|, 1) * E_z,     S = sum_z E_z,
    with min-weights built by two tensor_scalar ops (@4x) and the
    accumulation done as fp16 tensor_tensor (@2x) over a concatenated
    (val|wt) plane pair.  Engine split: PE matmuls, ACT does PSUM->fp16
    plane copies, GpSimd does the small transposed-panel copies + DMA,
    DVE does weights/accumulate/divide.
"""
import sys

import numpy as np
import ml_dtypes

sys.path.insert(0, "/opt/trn_rl_repo")

import concourse.bass as bass
import concourse.mybir as mybir
import concourse.tile as tile
import concourse.bacc as bacc
from concourse import bass_utils

F32 = mybir.dt.float32
F16 = mybir.dt.float16
ALU = mybir.AluOpType
AX = mybir.AxisListType

S = 8
NB = 16
H = W = 1024
GW = 129          # x cells
NCY = 68          # y-cell slots per half (67 used, padded)
NROW = 640        # padded rows per half (5 chunks of 128)
WP = 1032         # padded x: [-4, 1028)
OUT_OFF = 20      # local row of first output row
NCH = 5           # splat row chunks
NQ = 4            # slice row chunks (512 out rows)
MAGIC = 12582912.0  # 1.5 * 2**23
DEBUG = False
NPAN = 2 * NB       # 32 transposed panels per q: (p, E0|dE_m)


def _round_half_even_cells(rows):
    return np.round(rows.astype(np.float32) / np.float32(S)).astype(np.int64)


def _half_geometry(half):
    o0 = half * 512
    rows_out = np.arange(o0, o0 + 512)
    y0 = rows_out // S
    cyb0, cyb1 = int(y0.min()), int(y0.max() + 1)
    cyr0 = max(cyb0 - 2, 0)
    cyr1 = min(cyb1 + 2, GW - 1)
    return o0, cyr0, cyr1, cyb0, cyb1


def _host_inputs_for_half(img, fs, half):
    """img [1024,1024] f32 -> (padded rows [640,1032] f32,
    Sy [5,128,68] f16, LyGT [4,68,128] f16)."""
    o0, cyr0, cyr1, cyb0, cyb1 = _half_geometry(half)
    pad = np.full((NROW, WP), -1.0, np.float32)
    g0 = o0 - OUT_OFF
    glo, ghi = max(0, g0), min(H, g0 + NROW)
    pad[glo - g0:ghi - g0, 4:4 + W] = img[glo:ghi]

    sy = np.zeros((NCH, 128, NCY), np.float32)
    for c in range(NCH):
        loc = 128 * c + np.arange(128)
        g = g0 + loc
        valid = (g >= 0) & (g < H)
        cells = _round_half_even_cells(np.clip(g, 0, H - 1))
        ok = valid & (cells >= cyr0) & (cells <= cyr1)
        sy[c, np.arange(128)[ok], cells[ok] - cyr0] = 1.0

    ncyb = cyb1 - cyb0 + 1
    rows_out = np.arange(o0, o0 + 512)
    y0 = rows_out // S
    ty = (rows_out % S).astype(np.float32) / np.float32(S)
    Ly = np.zeros((512, ncyb), np.float32)
    Ly[np.arange(512), y0 - cyb0] = 1 - ty
    Ly[np.arange(512), y0 + 1 - cyb0] = ty
    Gy = np.zeros((ncyb, NCY), np.float32)
    for i in range(5):
        for cb in range(cyb0, cyb1 + 1):
            cr = cb + i - 2
            if cyr0 <= cr <= cyr1:
                Gy[cb - cyb0, cr - cyr0] += fs[i]
    LyG = (Ly @ Gy).astype(np.float32)          # [512, NCY]
    lygt = np.zeros((NQ, NCY, 128), np.float32)
    for q in range(NQ):
        lygt[q] = LyG[128 * q:128 * q + 128].T
    return pad, sy.astype(np.float16), lygt.astype(np.float16)


def _host_lx():
    """Lx [129,1024]: col = 8c+g reads (1-g/8)*V[c] + (g/8)*V[c+1].
    Returns (Lx rows 0..127 [128,1024], row 128 [1,1024])."""
    lx = np.zeros((GW, W), np.float32)
    cols = np.arange(W)
    c0 = cols // 8
    tx = (cols % 8) / 8.0
    lx[c0, cols] += 1.0 - tx
    c1 = np.minimum(c0 + 1, GW - 1)
    lx[c1, cols] += np.where(c0 + 1 <= GW - 1, tx, 0.0)
    return (lx[:128].astype(np.float16),
            lx[128:129].astype(np.float16))


def _ap(base, off_elems, free_pairs):
    """AP reusing base's partition pair with custom free dims (elem offsets)."""
    return bass.AP(base.tensor, base.offset + off_elems,
                   [list(base.ap[0])] + [list(p) for p in free_pairs])


def build_program(fs, fr):
    nc = bacc.Bacc(None, target_bir_lowering=False)
    halves = nc.dram_tensor("halves", [3, NROW, WP], F32, kind="ExternalInput")
    syd = nc.dram_tensor("sy", [3, NCH, 128, NCY], F16, kind="ExternalInput")
    lygtd = nc.dram_tensor("lygt", [3, NQ, NCY, 128], F16, kind="ExternalInput")
    lxd = nc.dram_tensor("lx", [128, W], F16, kind="ExternalInput")
    lx128d = nc.dram_tensor("lx128", [1, W], F16, kind="ExternalInput")
    outd = nc.dram_tensor("out", [3, 512, W], F32, kind="ExternalOutput")
    if DEBUG:
        dvx = nc.dram_tensor("dbg_vx", [NCY, 2 * NB * GW], F32, kind="ExternalOutput")
        dvbhd = nc.dram_tensor("dbg_vbh", [NCY, 2 * NB * GW], F16, kind="ExternalOutput")
        dvtd = nc.dram_tensor("dbg_vt", [128, NPAN * 128], F16, kind="ExternalOutput")
        dvt128d = nc.dram_tensor("dbg_vt128", [1, NPAN * 128], F16, kind="ExternalOutput")
        daccd = nc.dram_tensor("dbg_acc", [128, 2 * W], F16, kind="ExternalOutput")
        dfzhd = nc.dram_tensor("dbg_fzh", [128, W], F16, kind="ExternalOutput")
        debd = nc.dram_tensor("dbg_eb", [128, 2 * W], F16, kind="ExternalOutput")

    with tile.TileContext(nc) as tc:
        with (
            tc.tile_pool(name="img", bufs=2) as imgp,
            tc.tile_pool(name="code", bufs=5) as codep,    # rrh/dh per chunk
            tc.tile_pool(name="syp", bufs=6) as syp,
            tc.tile_pool(name="mask", bufs=4) as maskp,
            tc.tile_pool(name="ps", bufs=2, space="PSUM") as psp,
            tc.tile_pool(name="pst", bufs=2, space="PSUM") as pstp,
            tc.tile_pool(name="grid", bufs=1) as gridp,
            tc.tile_pool(name="gridh", bufs=1) as gridhp,
            tc.tile_pool(name="blur", bufs=1) as blurp,
            tc.tile_pool(name="vt", bufs=2) as vtp,
            tc.tile_pool(name="eb", bufs=3) as ebp,
            tc.tile_pool(name="wz", bufs=3) as wzp,
            tc.tile_pool(name="acc", bufs=2) as accp,
            tc.tile_pool(name="tmp", bufs=3) as tmpp,
            tc.tile_pool(name="cst", bufs=1) as cstp,
            tc.tile_pool(name="scr", bufs=2, space="DRAM") as scrp,
        ):
            lxt = cstp.tile([128, W], F16, tag="lx")
            nc.gpsimd.dma_start(lxt[:], lxd[:, :])
            lx128t = cstp.tile([1, W], F16, tag="lx128")
            nc.gpsimd.dma_start(lx128t[:], lx128d[:, :])

            for h in range(3):
                # ---------------- SPLAT ----------------
                rrhs, dhs, sys_ = [], [], []
                for c in range(NCH):
                    img = imgp.tile([128, WP], F32, tag="img")
                    nc.gpsimd.dma_start(img[:], halves[h, 128 * c:128 * c + 128, :])
                    fz32 = imgp.tile([128, WP], F32, tag="sfz")
                    nc.vector.tensor_scalar(fz32[:], img[:], 15.0, None, ALU.mult)
                    rr32 = imgp.tile([128, WP], F32, tag="sfz")
                    nc.vector.tensor_scalar(rr32[:], fz32[:], MAGIC, MAGIC,
                                            ALU.add, ALU.subtract)
                    rrh = codep.tile([128, WP], F16, tag="rrh")
                    nc.vector.tensor_copy(rrh[:], rr32[:])
                    dh = codep.tile([128, WP], F16, tag="dh")
                    nc.vector.tensor_tensor(dh[:], fz32[:], rr32[:], ALU.subtract)
                    syt = syp.tile([128, NCY], F16, tag="sy")
                    nc.gpsimd.dma_start(syt[:], syd[h, c])
                    rrhs.append(rrh); dhs.append(dh); sys_.append(syt)

                # VX: [68, (plane2, z16, cx129)]  plane0 = cnt, plane1 = D->val
                vx = gridp.tile([NCY, 2 * NB * GW], F32, tag="ga")

                for b in range(NB):
                    psC = psp.tile([NCY, WP], F32, tag="ps")
                    psD = psp.tile([NCY, WP], F32, tag="ps")
                    for c in range(NCH):
                        mh = maskp.tile([128, WP], F16, tag="pl")
                        nc.vector.tensor_scalar(mh[:], rrhs[c][:], float(b),
                                                None, ALU.is_equal)
                        vh = maskp.tile([128, WP], F16, tag="pl")
                        nc.vector.tensor_tensor(vh[:], mh[:], dhs[c][:], ALU.mult)
                        for (lo, hi) in ((0, 512), (512, 1024), (1024, 1032)):
                            nc.tensor.matmul(psC[:, lo:hi], sys_[c][:],
                                             mh[:, lo:hi], start=(c == 0),
                                             stop=(c == NCH - 1))
                            nc.tensor.matmul(psD[:, lo:hi], sys_[c][:],
                                             vh[:, lo:hi], start=(c == 0),
                                             stop=(c == NCH - 1))
                    for (p, ps) in ((0, psC), (1, psD)):
                        dst = _ap(vx[:, :], (p * NB + b) * GW, [[1, GW]])
                        src = _ap(ps[:, :], 0, [[8, GW], [1, 8]])
                        nc.vector.tensor_reduce(dst, src, AX.X, ALU.add)
                        corr = tmpp.tile([NCY, 64], F32, tag="corr")
                        nc.scalar.copy(corr[:], _ap(ps[:, :], 8, [[16, 64]]))
                        odd = _ap(vx[:, :], (p * NB + b) * GW + 1, [[2, 64]])
                        nc.vector.tensor_tensor(odd, odd, corr[:], ALU.subtract)
                        even = _ap(vx[:, :], (p * NB + b) * GW, [[2, 64]])
                        nc.vector.tensor_tensor(even, even, corr[:], ALU.add)

                # decode val_b = b*cnt_b + D_b (in place into plane 1)
                for b in range(NB):
                    cnt = _ap(vx[:, :], b * GW, [[1, GW]])
                    dsl = _ap(vx[:, :], (NB + b) * GW, [[1, GW]])
                    nc.vector.scalar_tensor_tensor(dsl, cnt, float(b), dsl,
                                                   ALU.mult, ALU.add)

                # fp16 grid + blurs (center tap first: no memset needed)
                vxh = gridhp.tile([NCY, 2 * NB * GW], F16, tag="gh0")
                nc.vector.tensor_copy(vxh[:], vx[:])

                vzh = gridhp.tile([NCY, 2 * NB * GW], F16, tag="gh1")
                for i in (2, 0, 1, 3, 4):           # z-blur, center first
                    sh = i - 2
                    z0r, z1r = max(0, -sh), NB - max(0, sh)
                    nzz = z1r - z0r
                    dst = _ap(vzh[:, :], z0r * GW,
                              [[NB * GW, 2], [GW, nzz], [1, GW]])
                    src = _ap(vxh[:, :], (z0r + sh) * GW,
                              [[NB * GW, 2], [GW, nzz], [1, GW]])
                    if i == 2:
                        nc.vector.tensor_scalar(dst, src, float(fr[i]), None,
                                                ALU.mult)
                    else:
                        t = blurp.tile([NCY, 2 * NB * GW], F16, tag="bt")
                        tdst = _ap(t[:, :], 0, [[NB * GW, 2], [GW, nzz], [1, GW]])
                        nc.vector.tensor_scalar(tdst, src, float(fr[i]), None,
                                                ALU.mult)
                        nc.vector.tensor_tensor(dst, dst, tdst, ALU.add)

                vbh = gridhp.tile([NCY, 2 * NB * GW], F16, tag="gh0")
                for i in (2, 0, 1, 3, 4):           # x-blur, center first
                    sh = i - 2
                    x0r, x1r = max(0, -sh), GW - max(0, sh)
                    nxx = x1r - x0r
                    dst = _ap(vbh[:, :], x0r, [[GW, 2 * NB], [1, nxx]])
                    src = _ap(vzh[:, :], x0r + sh, [[GW, 2 * NB], [1, nxx]])
                    if i == 2:
                        nc.vector.tensor_scalar(dst, src, float(fs[i]), None,
                                                ALU.mult)
                    else:
                        t = blurp.tile([NCY, 2 * NB * GW], F16, tag="bt")
                        tdst = _ap(t[:, :], 0, [[GW, 2 * NB], [1, nxx]])
                        nc.vector.tensor_scalar(tdst, src, float(fs[i]), None,
                                                ALU.mult)
                        nc.vector.tensor_tensor(dst, dst, tdst, ALU.add)
                # x-blur leaves dst cols outside [x0r,x1r) unwritten for
                # shifted taps only; center tap covered all of [0,GW) so
                # every column holds center + whatever shifted taps apply. OK.

                if DEBUG and h == 0:
                    nc.gpsimd.dma_start(dvx[:, :], vx[:])
                    nc.gpsimd.dma_start(dvbhd[:, :], vbh[:])
                # Delta grid along z: dvbh[p, m-1, c] = vbh[p,m,c]-vbh[p,m-1,c]
                dvbh = gridhp.tile([NCY, 2 * 15 * GW], F16, tag="dvh")
                nc.vector.tensor_tensor(
                    _ap(dvbh[:, :], 0, [[15 * GW, 2], [1, 15 * GW]]),
                    _ap(vbh[:, :], GW, [[NB * GW, 2], [1, 15 * GW]]),
                    _ap(vbh[:, :], 0, [[NB * GW, 2], [1, 15 * GW]]),
                    ALU.subtract)

                # ---------------- SLICE ----------------
                for q in range(NQ):
                    lygth = syp.tile([NCY, 128], F16, tag="lygt")
                    nc.gpsimd.dma_start(lygth[:], lygtd[h, q])
                    imgh = imgp.tile([128, W], F16, tag="imgo")
                    r0 = OUT_OFF + 128 * q
                    nc.gpsimd.dma_start(imgh[:], halves[h, r0:r0 + 128, 4:4 + W])
                    fzh = wzp.tile([128, W], F16, tag="fzh")
                    nc.vector.tensor_scalar(fzh[:], imgh[:], 15.0, None, ALU.mult)

                    # pass1-T: 32 transposed panels [cell,row]: per p,
                    # panel m=0 is E_0 (z=0 plane), m>=1 is Delta E_m.
                    vt = vtp.tile([128, NPAN * 128], F16, tag="vt")
                    for j in range(NPAN):
                        p, m = divmod(j, NB)
                        if m == 0:
                            stat = _ap(vbh[:, :], p * NB * GW, [[1, 128]])
                        else:
                            stat = _ap(dvbh[:, :], p * 15 * GW + (m - 1) * GW,
                                       [[1, 128]])
                        psT = pstp.tile([128, 128], F32, tag="pst")
                        nc.tensor.matmul(psT[:, :], stat, lygth[:],
                                         start=True, stop=True)
                        nc.scalar.copy(vt[:, 128 * j:128 * j + 128], psT[:, :])
                    # cell-128 column of every panel -> one flat partition
                    psT128 = pstp.tile([66, 128], F32, tag="pst")
                    stat = _ap(dvbh[:, :], 128, [[GW, 15]])
                    nc.tensor.matmul(psT128[0:15, :], stat, lygth[:],
                                     start=True, stop=True)
                    stat = _ap(dvbh[:, :], 15 * GW + 128, [[GW, 15]])
                    nc.tensor.matmul(psT128[32:47, :], stat, lygth[:],
                                     start=True, stop=True)
                    stat = _ap(vbh[:, :], 128, [[NB * GW, 2]])
                    nc.tensor.matmul(psT128[64:66, :], stat, lygth[:],
                                     start=True, stop=True)
                    s128 = vtp.tile([66, 128], F16, tag="s128")
                    nc.scalar.copy(s128[:], psT128[:])
                    scr = scrp.tile([66, 128], F16, tag="scr")
                    nc.gpsimd.dma_start(scr[:], s128[:])
                    vt128 = vtp.tile([1, NPAN * 128], F16, tag="vt128")
                    nc.gpsimd.dma_start(
                        vt128[0:1, 0:30 * 128],
                        bass.AP(scr.tensor, scr.offset,
                                [[32 * 128, 2], [128, 15], [1, 128]]))
                    nc.gpsimd.dma_start(vt128[0:1, 30 * 128:32 * 128],
                                        scr[64:66, :])

                    if DEBUG and h == 0 and q == 0:
                        nc.gpsimd.dma_start(dvtd[:, :], vt[:])
                        nc.gpsimd.dma_start(dvt128d[:, :], vt128[:])
                        nc.gpsimd.dma_start(dfzhd[:, :], fzh[:])

                    def pass2(j, dst_f16):
                        """PE x-expand panel j into psE then ACT-copy to f16."""
                        p, m = divmod(j, NB)
                        slot = 15 * p + (m - 1) if m >= 1 else 30 + p
                        psE = psp.tile([128, W], F32, tag="ps")
                        stat = vt[:, 128 * j:128 * j + 128]
                        nc.tensor.matmul(psE[:, 0:512], stat, lxt[:, 0:512],
                                         start=True, stop=True)
                        nc.tensor.matmul(psE[:, 512:1016], stat,
                                         lxt[:, 512:1016], start=True,
                                         stop=True)
                        nc.tensor.matmul(psE[:, 1016:1024], stat,
                                         lxt[:, 1016:1024], start=True,
                                         stop=False)
                        nc.tensor.matmul(psE[:, 1016:1024],
                                         vt128[0:1, 128 * slot:128 * slot + 128],
                                         lx128t[:, 1016:1024], start=False,
                                         stop=True)
                        nc.scalar.copy(dst_f16, psE[:, :])

                    # acc init from E_0 panels (m=0)
                    accb = accp.tile([128, 2 * W], F16, tag="accb")
                    pass2(0, accb[:, 0:W])
                    pass2(NB, accb[:, W:2 * W])

                    for m in range(1, NB):
                        a = wzp.tile([128, W], F16, tag="wz")
                        nc.vector.tensor_scalar(a[:], fzh[:], float(m - 1), 0.0,
                                                ALU.subtract, ALU.max)
                        r = wzp.tile([128, W], F16, tag="wz")
                        nc.vector.tensor_scalar(r[:], a[:], 1.0, None, ALU.min)
                        eb = ebp.tile([128, 2 * W], F16, tag="eb")
                        pass2(m, eb[:, 0:W])
                        pass2(NB + m, eb[:, W:2 * W])
                        tmp = ebp.tile([128, 2 * W], F16, tag="tmul")
                        wrep = _ap(r[:, :], 0, [[0, 2], [1, W]])
                        nc.vector.tensor_tensor(tmp[:], wrep, eb[:], ALU.mult)
                        nc.vector.tensor_tensor(accb[:], accb[:], tmp[:],
                                                ALU.add)
                        if DEBUG and h == 0 and q == 0 and m == 1:
                            nc.gpsimd.dma_start(debd[:, :], eb[:])

                    if DEBUG and h == 0 and q == 0:
                        nc.gpsimd.dma_start(daccd[:, :], accb[:])
                    den = tmpp.tile([128, W], F32, tag="sc2")
                    nc.vector.tensor_scalar(den[:], accb[:, 0:W], 15.0,
                                            1.5e-7, ALU.mult, ALU.add)
                    rec = tmpp.tile([128, W], F32, tag="sc2")
                    scr = tmpp.tile([128, W], F32, tag="sc2")
                    nc.vector.reciprocal_approx_accurate(rec[:], den[:], scr[:])
                    res = tmpp.tile([128, W], F32, tag="sc2")
                    nc.vector.tensor_tensor(res[:], accb[:, W:2 * W], rec[:],
                                            ALU.mult)
                    nc.gpsimd.dma_start(outd[h, 128 * q:128 * q + 128, :],
                                        res[:])
    nc.finalize()
    return nc


_PROGRAM_CACHE = {}


def _cached_program(fs, fr):
    key = (tuple(np.asarray(fs, np.float32).tolist()),
           tuple(np.asarray(fr, np.float32).tolist()))
    if key not in _PROGRAM_CACHE:
        _PROGRAM_CACHE[key] = build_program(np.asarray(fs, np.float32),
                                            np.asarray(fr, np.float32))
    return _PROGRAM_CACHE[key]


def kernel(blurred_batch, kernel_batch, filter_s, filter_r,
           num_irls_iter=None, num_cg_iter=None):
    imgs = np.asarray(blurred_batch, np.float32).reshape(12, H, W)
    fs = np.asarray(filter_s, np.float32)
    fr = np.asarray(filter_r, np.float32)

    lx, lx128 = _host_lx()
    nc = _cached_program(fs, fr)

    in_maps = []
    for core in range(8):
        hv = np.zeros((3, NROW, WP), np.float32)
        sy = np.zeros((3, NCH, 128, NCY), np.float16)
        ly = np.zeros((3, NQ, NCY, 128), np.float16)
        for s in range(3):
            g = 3 * core + s
            pad, syh, lygt = _host_inputs_for_half(imgs[g // 2], fs, g % 2)
            hv[s], sy[s], ly[s] = pad, syh, lygt
        in_maps.append({"halves": hv, "sy": sy, "lygt": ly,
                        "lx": lx, "lx128": lx128})

    res = bass_utils.run_bass_kernel_spmd(nc, in_maps, core_ids=list(range(8)))
    out = np.zeros((12, H, W), np.float32)
    for core in range(8):
        o = res.results[core]["out"]
        for s in range(3):
            g = 3 * core + s
            out[g // 2, (g % 2) * 512:(g % 2) * 512 + 512] = o[s]
    return out.reshape(4, 3, H, W)
